# revision 12
# baseline (speedup 1.0000x reference)
"""4-layer GCN (GCNConv+ReLU x4, 128-64-32-64-128) on 8 Trainium2 NeuronCores.

Strategy (dst-sharded message passing):
  - Host: deg/norm precompute. out[d] = s[d]*(sum_{src->d} h'[src] + h'[d]) + b
    with h' = (s*x) @ W and s = deg^-1/2 -- so no per-edge scaling is needed.
  - dst nodes sharded across 8 cores (12500 each), degree-sorted into tiles
    of 128. Edges sorted by (dst tile, src class), padded to 128-multiples
    per (tile, class) with zero-row indices; per-(tile,class) column
    counts are maxed across cores so all cores run one SPMD program.
  - The h' exchange table packs `pack = 128//Fo` nodes per 256B row, so the
    AllGather moves only Fo-wide rows. Gather classes are (window, parity):
    window = 25008-packed-row range (int16 idx limit), parity = rank % pack
    (selects the Fo-slice within the gathered 256B row). Layers 1 and 3
    (Fo=64) share one edge layout; layer 2 (Fo=32) and layer 4 (Fo=128)
    have their own.
  - Device, per layer: h' = x~^T @ W (PE, node-major tiles) -> packed bf16
    shard -> AllGather table in DRAM -> dma_gather 256B rows per edge on 4
    SWDGE queues (one per class) -> one-hot chunk matrices (DVE is_equal
    against iota, batched) -> PE matmul segment-sum accumulated in PSUM
    (+ identity-matmul self term) -> ReLU/scale epilogue.
  - Layers 1-3 keep x~ feature-major in SBUF; layer 4 emits node-major
    output tiles, DMA'd out; host inverse-permutes.
"""

import numpy as np

# ---------------------------------------------------------------------------
# configuration
# ---------------------------------------------------------------------------

P = 128
FPAD = 128          # gathered row elements (bf16 -> 256B rows)
NCORES = 8
GROUP_TILES = 7     # dst tiles per gather group
MB = 8              # M-matrix build batch (chunks per DVE op)
NCLS = 4            # gather classes per layer = window x parity
WROWS = 25008       # packed rows per idx window (8 * 12504 // 4)

# layer variants: pack = nodes per 256B table row = 128 // Fo
VARIANTS = (2, 4, 1)            # pack for variant 0,1,2
LAYER_VAR = (0, 1, 0, 2)        # layer -> variant (Fo = 64, 32, 64, 128)


class Cfg:
    def __init__(self, n_nodes, channels=(128, 64, 32, 64, 128)):
        self.N = n_nodes
        self.NPC = n_nodes // NCORES            # 12500
        self.NTILES = (self.NPC + P - 1) // P   # 98
        self.NPAD = self.NTILES * P             # 12544
        self.SHARD_NODES = self.NPC + 4         # + zero pad nodes -> 12504
        self.channels = channels
        self.dims = list(zip(channels[:-1], channels[1:]))


FULL = Cfg(100000)

# ---------------------------------------------------------------------------
# host preprocessing
# ---------------------------------------------------------------------------


def _variant_streams(src_core, src_rank, dst_core, dst_tile, dst_slot,
                     pack, cfg: Cfg):
    """Edge layout for one pack variant. Returns per-core idx/dstloc streams
    (int32 / int32) in global-column order plus column metadata."""
    NPC, NTILES, SN = cfg.NPC, cfg.NTILES, cfg.SHARD_NODES
    rows_per_core = SN // pack
    packed_row = src_core * rows_per_core + src_rank // pack
    parity = src_rank % pack
    window = packed_row // WROWS
    cls = window * pack + parity
    idx_local = packed_row - window * WROWS
    # zero slot per class: core (4w // pack ... ) -- node NPC+j of the first
    # core in window w has packed row w*WROWS + NPC//pack, parity j.
    zero_idx = NPC // pack      # same value for every window

    key = (dst_core * NTILES + dst_tile) * NCLS + cls
    order = np.argsort(key, kind="stable")
    key_s = key[order]
    idx_s = idx_local[order].astype(np.int32)
    dst_slot_s = dst_slot[order].astype(np.int32)

    counts = np.bincount(key_s, minlength=NCORES * NTILES * NCLS)
    counts = counts.reshape(NCORES, NTILES, NCLS)
    kcols = np.ceil(counts.max(axis=0) / P).astype(np.int64)  # [NTILES, NCLS]

    ngroups = (NTILES + GROUP_TILES - 1) // GROUP_TILES
    col_off = np.zeros((NTILES, NCLS), dtype=np.int64)
    group_col0 = np.zeros(ngroups + 1, dtype=np.int64)
    segs = []                                    # (g, cls, col0, ncols)
    acc = 0
    for g in range(ngroups):
        group_col0[g] = acc
        t0, t1 = g * GROUP_TILES, min((g + 1) * GROUP_TILES, NTILES)
        for q in range(NCLS):
            c0 = acc
            for t in range(t0, t1):
                col_off[t, q] = acc
                acc += int(kcols[t, q])
            if acc > c0:
                segs.append((g, q, int(c0), int(acc - c0)))
    group_col0[ngroups] = acc
    total_cols = acc
    stream_len = total_cols * P

    idx16 = np.zeros((NCORES, stream_len), dtype=np.int32)
    # default padding: zero slot of the column's class (filled below per seg)
    dstloc = np.zeros((NCORES, stream_len), dtype=np.int32)
    starts = np.zeros(NCORES * NTILES * NCLS + 1, dtype=np.int64)
    np.cumsum(np.bincount(key_s, minlength=NCORES * NTILES * NCLS),
              out=starts[1:])
    # fill padding with the zero-slot index (class-independent value)
    idx16[:] = zero_idx
    for c in range(NCORES):
        base = c * NTILES * NCLS
        for t in range(NTILES):
            for q in range(NCLS):
                k = base + t * NCLS + q
                a, b = starts[k], starts[k + 1]
                if b > a:
                    pos0 = col_off[t, q] * P
                    idx16[c, pos0:pos0 + (b - a)] = idx_s[a:b]
                    dstloc[c, pos0:pos0 + (b - a)] = dst_slot_s[a:b]
    assert idx16.max(initial=0) <= 32767

    # tile-major column order (for dstloc / M batches)
    tm_of = np.zeros(total_cols, dtype=np.int64)
    tile_tm0 = np.zeros(NTILES + 1, dtype=np.int64)
    tm = 0
    for t in range(NTILES):
        tile_tm0[t] = tm
        for q in range(NCLS):
            o = int(col_off[t, q])
            for j in range(int(kcols[t, q])):
                tm_of[o + j] = tm
                tm += 1
    tile_tm0[NTILES] = tm
    assert tm == total_cols

    return dict(pack=pack, kcols=kcols, col_off=col_off,
                total_cols=total_cols, stream_len=stream_len,
                ngroups=ngroups, segs=segs, group_col0=group_col0,
                tm_of=tm_of, tile_tm0=tile_tm0, idx16=idx16, dstloc=dstloc)


def preprocess(edge_index, cfg: Cfg):
    src = np.asarray(edge_index[0], dtype=np.int64)
    dst = np.asarray(edge_index[1], dtype=np.int64)
    N, NPC, NTILES = cfg.N, cfg.NPC, cfg.NTILES

    deg = np.bincount(dst, minlength=N).astype(np.float32) + 1.0
    deg_isqrt = (1.0 / np.sqrt(deg)).astype(np.float32)

    node_order = np.empty((NCORES, cfg.NPAD), dtype=np.int64)
    node_valid = np.zeros((NCORES, cfg.NPAD), dtype=bool)
    rank_of = np.empty(N, dtype=np.int64)
    for c in range(NCORES):
        nodes = np.arange(c * NPC, (c + 1) * NPC)
        perm = nodes[np.argsort(deg[nodes], kind="stable")]
        node_order[c, :NPC] = perm
        node_order[c, NPC:] = perm[-1] if NPC else 0
        node_valid[c, :NPC] = True
        rank_of[perm] = np.arange(NPC)

    src_core = src // NPC
    src_rank = rank_of[src]
    dst_core = dst // NPC
    dst_rank = rank_of[dst]
    dst_tile = dst_rank // P
    dst_slot = dst_rank % P

    variants = [
        _variant_streams(src_core, src_rank, dst_core, dst_tile, dst_slot,
                         pack, cfg)
        for pack in VARIANTS
    ]

    meta = dict(cfg=cfg, variants=variants, node_order=node_order,
                node_valid=node_valid, deg_isqrt=deg_isqrt)
    return None, None, meta

# ---------------------------------------------------------------------------
# device program
# ---------------------------------------------------------------------------


def build_program(meta, with_bias):
    import concourse.bass as bass
    import concourse.bacc as bacc
    import concourse.tile as tile
    from concourse import mybir

    cfg: Cfg = meta["cfg"]
    NT, NPAD, SN = cfg.NTILES, cfg.NPAD, cfg.SHARD_NODES
    variants = meta["variants"]
    dims = cfg.dims
    NL = len(dims)
    f32, bf16, i16 = mybir.dt.float32, mybir.dt.bfloat16, mybir.dt.int16
    AF = mybir.ActivationFunctionType
    OP = mybir.AluOpType

    nc = bacc.Bacc("TRN2", target_bir_lowering=False, debug=False,
                   num_devices=NCORES, num_swdge_queues=4)

    # ---- I/O ----
    xT_d = nc.dram_tensor("xT", [dims[0][0], NPAD], f32, kind="ExternalInput")
    idx_d = [nc.dram_tensor(f"idx16_{v}", [P, variants[v]["stream_len"] // 16],
                            i16, kind="ExternalInput") for v in range(3)]
    dstloc_d = [nc.dram_tensor(f"dstloc_{v}", [P, variants[v]["total_cols"]],
                               bf16, kind="ExternalInput") for v in range(3)]
    scol2_d = nc.dram_tensor("scol2", [P, NT], f32, kind="ExternalInput")
    scol1_d = nc.dram_tensor("scol1", [P, NT], f32, kind="ExternalInput")
    identf_d = nc.dram_tensor("identf", [P, P], f32, kind="ExternalInput")
    iota_d = nc.dram_tensor("iota", [P, P], bf16, kind="ExternalInput")
    ident_d = nc.dram_tensor("ident", [P, P], bf16, kind="ExternalInput")
    W_d = [nc.dram_tensor(f"W{l+1}", [dims[l][0], FPAD], f32,
                          kind="ExternalInput") for l in range(NL)]
    sbias_d = [nc.dram_tensor(f"sbias{l+1}", [NPAD, dims[l][1]], f32,
                              kind="ExternalInput") if with_bias else None
               for l in range(NL)]
    out_d = nc.dram_tensor("out", [NPAD, dims[-1][1]], f32,
                           kind="ExternalOutput")

    # packed h' shard / table per layer: Fo-wide rows
    shard_d = [nc.dram_tensor(f"shard{l}", [SN, dims[l][1]], bf16)
               for l in range(NL)]
    table_d = [nc.dram_tensor(f"table{l}",
                              [1, NCORES * SN * dims[l][1] + 2 * FPAD], bf16,
                              addr_space="Shared")
               for l in range(NL)]

    with tile.TileContext(nc) as tc:
        import contextlib
        ctx = contextlib.ExitStack()
        with ctx:
            pers = ctx.enter_context(tc.tile_pool(name="pers", bufs=1))
            msgs_pool = ctx.enter_context(tc.tile_pool(name="msgs", bufs=2))
            idx_pool = ctx.enter_context(tc.tile_pool(name="idxp", bufs=2))
            m_pool = ctx.enter_context(tc.tile_pool(name="mmat", bufs=8))
            tmp_pool = ctx.enter_context(tc.tile_pool(name="tmp", bufs=4))
            psum_h = ctx.enter_context(
                tc.tile_pool(name="psum_h", bufs=2, space="PSUM"))
            psum_o = ctx.enter_context(
                tc.tile_pool(name="psum_o", bufs=4, space="PSUM"))

            # ---- persistent SBUF ----
            xT = pers.tile([P, NPAD], f32, tag="xT")
            identf = pers.tile([P, P], f32, tag="identf")
            dstloc_sb = [pers.tile([P, variants[v]["total_cols"]], bf16,
                                   tag=f"dstloc{v}", name=f"dstloc{v}")
                         for v in range(3)]
            scol2 = pers.tile([P, NT], f32, tag="scol2")
            scol1 = pers.tile([P, NT], f32, tag="scol1")
            iota = pers.tile([P, P], bf16, tag="iota")
            ident = pers.tile([P, P], bf16, tag="ident")
            hnode = pers.tile([P, NT * FPAD], bf16, tag="hnode")
            zrow = pers.tile([4, FPAD], bf16, tag="zrow")
            W_sb = [pers.tile([dims[l][0], FPAD], f32, tag=f"W{l}",
                              name=f"Wsb{l}") for l in range(NL)]

            nc.sync.dma_start(xT[:], xT_d[:, :])
            for v in range(3):
                nc.sync.dma_start(dstloc_sb[v][:], dstloc_d[v][:, :])
            nc.sync.dma_start(scol2[:], scol2_d[:, :])
            nc.sync.dma_start(scol1[:], scol1_d[:, :])
            nc.sync.dma_start(identf[:], identf_d[:, :])
            nc.sync.dma_start(iota[:], iota_d[:, :])
            nc.sync.dma_start(ident[:], ident_d[:, :])
            for l in range(NL):
                nc.sync.dma_start(W_sb[l][:], W_d[l][:, :])
            nc.gpsimd.memset(zrow[:], 0.0)

            for l in range(NL):
                Fi, Fo = dims[l]
                last_layer = l == NL - 1
                v = LAYER_VAR[l]
                var = variants[v]
                pack = var["pack"]
                kcols, col_off = var["kcols"], var["col_off"]
                segs, group_col0 = var["segs"], var["group_col0"]
                ngroups = var["ngroups"]
                tm_of, tile_tm0 = var["tm_of"], var["tile_tm0"]

                # ---- phase 1: h' = x~^T @ W, node-major bf16 tiles ----
                for t in range(NT):
                    csl = slice(t * P, (t + 1) * P)
                    ph = psum_h.tile([P, FPAD], f32, tag="ph")
                    nc.tensor.matmul(ph[:], lhsT=xT[0:Fi, csl],
                                     rhs=W_sb[l][:, :], start=True, stop=True)
                    hsl = hnode[:, t * FPAD:(t + 1) * FPAD]
                    nc.scalar.activation(hsl, ph[:], AF.Identity)
                    # packed shard rows for this tile (Fo-wide)
                    r0 = t * P
                    nrows = min(P, cfg.NPC - r0)
                    if nrows > 0:
                        nc.sync.dma_start(
                            shard_d[l][r0:r0 + nrows, :],
                            hnode[0:nrows, t * FPAD:t * FPAD + Fo])
                nc.sync.dma_start(shard_d[l][cfg.NPC:SN, :], zrow[:, 0:Fo])

                # ---- phase 2: AllGather the packed table ----
                nc.gpsimd.collective_compute(
                    "AllGather", OP.bypass,
                    replica_groups=[list(range(NCORES))],
                    ins=[shard_d[l][:, :]],
                    outs=[table_d[l][0, 0:NCORES * SN * Fo]],
                )
                # 256B-row view of the table for gathering
                t2d = table_d[l][0, :].rearrange("(r e) -> r e", e=FPAD)

                # ---- phase 3: gather + segment-sum per group ----
                for g in range(ngroups):
                    t0 = g * GROUP_TILES
                    t1 = min(t0 + GROUP_TILES, NT)
                    gc0, gc1 = int(group_col0[g]), int(group_col0[g + 1])
                    gcols = gc1 - gc0
                    if gcols == 0:
                        continue
                    msgs = msgs_pool.tile([P, gcols * FPAD], bf16, tag="msgs")
                    m3 = msgs[:].rearrange("p (k f) -> p k f", f=FPAD)
                    idxg = idx_pool.tile([P, gcols * 8], i16, tag="idxg")
                    nc.sync.dma_start(idxg[:],
                                      idx_d[v][:, gc0 * 8:gc1 * 8])
                    for (sg, sq, c0, ncols) in segs:
                        if sg != g:
                            continue
                        w = sq // pack
                        # window base in elements; parity picks the Fo-slice
                        # of each gathered 256B row at matmul time
                        ebase = w * WROWS * FPAD
                        win = table_d[l][0, ebase:ebase + WROWS * FPAD] \
                            .rearrange("(r e) -> r e", e=FPAD)
                        nidx = ncols * P
                        nc.gpsimd.dma_gather(
                            m3[:, c0 - gc0:c0 - gc0 + ncols, :],
                            win,
                            idxg[:, (c0 - gc0) * 8:(c0 - gc0 + ncols) * 8],
                            nidx, nidx, FPAD,
                            single_packet=nidx <= 1024,
                            queue_num=sq,
                        )
                    for t in range(t0, t1):
                        # (column, parity-offset) runs for this tile
                        cols = []
                        for q in range(NCLS):
                            o = int(col_off[t, q])
                            joff = (q % pack) * Fo
                            cols += [(c, joff)
                                     for c in range(o, o + int(kcols[t, q]))]
                        # per-tile M batches over tile-major dstloc columns
                        ttm0, ttm1 = int(tile_tm0[t]), int(tile_tm0[t + 1])
                        mbats = []
                        for mb0 in range(ttm0, ttm1, MB):
                            nb = min(MB, ttm1 - mb0)
                            mt = m_pool.tile([P, MB * P], bf16, tag="m")
                            din = dstloc_sb[v][:, mb0:mb0 + nb]
                            din3 = din[:, :, None].to_broadcast([P, nb, P])
                            io3 = iota[:, None, :].to_broadcast([P, nb, P])
                            mt3 = mt[:].rearrange("p (k f) -> p k f", f=P)
                            nc.vector.tensor_tensor(mt3[:, 0:nb, :], din3,
                                                    io3, op=OP.is_equal)
                            mbats.append((mb0, nb, mt))

                        def mslice(c):
                            tmc = int(tm_of[c])
                            for (mb0, nb, mt) in mbats:
                                if mb0 <= tmc < mb0 + nb:
                                    return mt[:, (tmc - mb0) * P:
                                              (tmc - mb0 + 1) * P]
                            raise AssertionError
                        po = psum_o.tile([P, Fo], f32, tag="po")
                        for i, (c, joff) in enumerate(cols):
                            nc.tensor.matmul(
                                po[:], lhsT=mslice(c),
                                rhs=m3[:, c - gc0, joff:joff + Fo],
                                start=i == 0, stop=False)
                        hsl = hnode[:, t * FPAD:t * FPAD + Fo]
                        nc.tensor.matmul(po[:], lhsT=ident[:], rhs=hsl,
                                         start=len(cols) == 0, stop=True)

                        # ---- epilogue (node-major) ----
                        csl = slice(t * P, (t + 1) * P)
                        scol = scol2 if not last_layer else scol1
                        ot = tmp_pool.tile([P, Fo], f32, tag="otile")
                        if with_bias:
                            sb_t = tmp_pool.tile([P, Fo], f32, tag="sbias")
                            nc.sync.dma_start(
                                sb_t[:], sbias_d[l][t * P:(t + 1) * P, :])
                            t1b = tmp_pool.tile([P, Fo], f32, tag="tmpb")
                            nc.vector.scalar_tensor_tensor(
                                t1b[:], po[:], scol[:, t:t + 1], sb_t[:],
                                op0=OP.mult, op1=OP.add)
                            nc.scalar.activation(ot[:], t1b[:], AF.Relu)
                        else:
                            nc.scalar.activation(ot[:], po[:], AF.Relu,
                                                 scale=scol[:, t:t + 1])
                        if not last_layer:
                            pt = psum_h.tile([Fo, P], f32, tag="ptr")
                            nc.tensor.transpose(pt[:], ot[:], identf[:])
                            nc.scalar.activation(xT[0:Fo, csl], pt[:],
                                                 AF.Identity)
                        else:
                            nc.sync.dma_start(out_d[t * P:(t + 1) * P, :],
                                              ot[:])

    nc.compile()
    return nc

# ---------------------------------------------------------------------------
# runtime glue
# ---------------------------------------------------------------------------


def _bf16(a):
    import ml_dtypes
    return np.asarray(a, dtype=np.float32).astype(ml_dtypes.bfloat16)


def build_inputs(x, Ws, bs, idx16, dstloc, meta, with_bias):
    cfg: Cfg = meta["cfg"]
    node_order, node_valid = meta["node_order"], meta["node_valid"]
    deg_isqrt = meta["deg_isqrt"]
    variants = meta["variants"]
    x = np.asarray(x, dtype=np.float32)

    # per-variant idx streams wrapped [16, L/16], replicated to 128
    # partitions (8 copies: 4 queues x 2 Q7 cores read their own band)
    idxw_v, dstloc_v = [], []
    for var in variants:
        stream_len, total_cols = var["stream_len"], var["total_cols"]
        idxw = np.zeros((NCORES, 16, stream_len // 16), dtype=np.int16)
        for c in range(NCORES):
            for (_g, _q, c0, ncols) in var["segs"]:
                seg = var["idx16"][c, c0 * P:(c0 + ncols) * P]
                idxw[c, :, c0 * 8:(c0 + ncols) * 8] = (
                    seg.reshape(-1, 16).T.astype(np.int16))
        idxw_v.append(np.tile(idxw, (1, 8, 1)))

        dl = var["dstloc"].reshape(NCORES, total_cols, P)
        dl_tm = np.zeros_like(dl)
        dl_tm[:, var["tm_of"], :] = dl          # tile-major column order
        dstloc_v.append(_bf16(np.transpose(dl_tm, (0, 2, 1))))

    iota = _bf16(np.broadcast_to(np.arange(P, dtype=np.float32), (P, P)))
    ident = _bf16(np.eye(P, dtype=np.float32))

    in_maps = []
    for c in range(NCORES):
        nodes = node_order[c]
        valid = node_valid[c]
        s1 = np.where(valid, deg_isqrt[nodes], 0.0).astype(np.float32)
        xt = (x[nodes] * s1[:, None]).astype(np.float32)     # [NPAD, Fin]
        m = {
            "xT": np.ascontiguousarray(xt.T),                # [Fin, NPAD]
            "scol2": np.ascontiguousarray(
                (s1 * s1).reshape(cfg.NTILES, P).T).astype(np.float32),
            "scol1": np.ascontiguousarray(
                s1.reshape(cfg.NTILES, P).T).astype(np.float32),
            "iota": iota,
            "ident": ident,
            "identf": np.eye(P, dtype=np.float32),
        }
        for v in range(3):
            m[f"idx16_{v}"] = idxw_v[v][c]
            m[f"dstloc_{v}"] = dstloc_v[v][c]
        for l in range(len(Ws)):
            W = np.asarray(Ws[l], dtype=np.float32)
            Wp = np.zeros((W.shape[0], FPAD), dtype=np.float32)
            Wp[:, :W.shape[1]] = W
            m[f"W{l+1}"] = Wp
            if with_bias:
                b = np.asarray(bs[l], dtype=np.float32)
                if l < len(Ws) - 1:
                    m[f"sbias{l+1}"] = np.ascontiguousarray(
                        s1[:, None] * b[None, :]).astype(np.float32)
                else:
                    m[f"sbias{l+1}"] = np.ascontiguousarray(
                        np.broadcast_to(b[None, :], (cfg.NPAD, len(b)))
                    ).astype(np.float32)
        in_maps.append(m)
    return in_maps


def assemble_output(results, meta, n_out_feats):
    cfg: Cfg = meta["cfg"]
    node_order, node_valid = meta["node_order"], meta["node_valid"]
    full = np.zeros((cfg.N, n_out_feats), dtype=np.float32)
    for c in range(NCORES):
        out_c = np.asarray(results[c]["out"], dtype=np.float32)
        full[node_order[c, :cfg.NPC]] = out_c[:cfg.NPC]
    return full


_PROGRAM_CACHE = {}


def run(x, edge_index, Ws, bs, cfg):
    from concourse.bass_utils import run_bass_kernel_spmd

    idx16, dstloc, meta = preprocess(edge_index, cfg)
    with_bias = any(np.any(np.asarray(b)) for b in bs)

    key = (cfg.N, tuple(cfg.channels), with_bias,
           tuple(v["total_cols"] for v in meta["variants"]),
           tuple(tuple(np.asarray(v["kcols"]).ravel())
                 for v in meta["variants"]))
    if key not in _PROGRAM_CACHE:
        _PROGRAM_CACHE[key] = build_program(meta, with_bias)
    nc = _PROGRAM_CACHE[key]

    in_maps = build_inputs(x, Ws, bs, idx16, dstloc, meta, with_bias)
    res = run_bass_kernel_spmd(nc, in_maps, list(range(NCORES)))
    return assemble_output(res.results, meta, cfg.channels[-1])


def kernel(x, edge_index, W1, b1, W2, b2, W3, b3, W4, b4):
    return run(x, edge_index, [W1, W2, W3, W4], [b1, b2, b3, b4], FULL)
